# revision 4
# baseline (speedup 1.0000x reference)
"""MoE router kernel for TRN2, 8 NeuronCores, data-parallel over tokens.

reference computation (per problem spec, T=16384, H=4096, E=8, K=2):
    logits  = hidden @ gate_w.T            [T, E]
    probs   = softmax(logits, -1)
    rw, sel = top_k(probs, 2); rw /= rw.sum(-1, keepdims=True)
    f = mean(one_hot(sel[:, 0], E)); P = mean(probs, 0)
    aux = 0.01 * E * sum(f * P)
    returns (rw, sel, logits, aux)

Sharding: tokens split 8 ways (2048/core); gate replicated; f/P partial sums
reduced on host (8x16 floats).
"""

import numpy as np

import concourse.bacc as bacc
import concourse.bass as bass
import concourse.mybir as mybir
from concourse import bass_utils
from concourse.tile import TileContext

T, H, E, TOPK = 16384, 4096, 8, 2
AUX_COEF = 0.01
N_CORES = 8
T_LOC = T // N_CORES          # 2048 tokens per core
P = 128                       # partitions
N_TILES = T_LOC // P          # 16 token tiles per core
N_CHUNKS = H // P             # 32 h-chunks
CHUNKS_PER_GRP = 4            # transpose group -> [128, 512] psum bank
N_GRPS = N_CHUNKS // CHUNKS_PER_GRP

FP32 = mybir.dt.float32
U32 = mybir.dt.uint32
I32 = mybir.dt.int32


def _build():
    nc = bacc.Bacc("TRN2", target_bir_lowering=False, debug=False,
                   num_devices=N_CORES)

    hid = nc.dram_tensor("hid", [T_LOC, H], FP32, kind="ExternalInput")
    # gate pre-arranged on host to [P, N_CHUNKS*E]: [p, (c e)] = gate_w.T[c*128+p, e]
    gate_t = nc.dram_tensor("gate_t", [P, N_CHUNKS * E], FP32, kind="ExternalInput")
    ident = nc.dram_tensor("ident", [P, P], FP32, kind="ExternalInput")
    iota8 = nc.dram_tensor("iota8", [P, E], FP32, kind="ExternalInput")
    ones = nc.dram_tensor("ones", [P, 1], FP32, kind="ExternalInput")

    logits_out = nc.dram_tensor("logits_out", [T_LOC, E], FP32, kind="ExternalOutput")
    rw_out = nc.dram_tensor("rw_out", [T_LOC, TOPK], FP32, kind="ExternalOutput")
    sel_out = nc.dram_tensor("sel_out", [T_LOC, TOPK], I32, kind="ExternalOutput")
    fp_out = nc.dram_tensor("fp_out", [1, 2 * E], FP32, kind="ExternalOutput")

    with TileContext(nc) as tc:
        with (
            tc.tile_pool(name="const", bufs=1) as cpool,
            tc.tile_pool(name="hid", bufs=3) as hid_pool,
            tc.tile_pool(name="hidT", bufs=4) as hidT_pool,
            tc.tile_pool(name="tail", bufs=4) as tail_pool,
            tc.tile_pool(name="outs", bufs=4) as out_pool,
            tc.tile_pool(name="tp_ps", bufs=2, space="PSUM") as tp_psum,
            tc.tile_pool(name="lg_ps", bufs=3, space="PSUM") as lg_psum,
            tc.tile_pool(name="st_ps", bufs=1, space="PSUM") as st_psum,
        ):
            gate_sb = cpool.tile([P, N_CHUNKS * E], FP32)
            nc.sync.dma_start(gate_sb[:, :], gate_t[:, :])
            ident_sb = cpool.tile([P, P], FP32)
            nc.sync.dma_start(ident_sb[:, :], ident[:, :])
            iota_sb = cpool.tile([P, E], FP32)
            nc.sync.dma_start(iota_sb[:, :], iota8[:, :])
            ones_sb = cpool.tile([P, 1], FP32)
            nc.sync.dma_start(ones_sb[:, :], ones[:, :])

            # stats accumulator: cols 0:8 one-hot(top1) sums, 8:16 prob sums
            stat_acc = cpool.tile([P, 2 * E], FP32)
            nc.vector.memset(stat_acc[:, :], 0.0)

            for i in range(N_TILES):
                hid_t = hid_pool.tile([P, H], FP32)
                nc.sync.dma_start(hid_t[:, :], hid[i * P:(i + 1) * P, :])

                lg_ps = lg_psum.tile([P, E], FP32)
                for g in range(N_GRPS):
                    tp_ps = tp_psum.tile([P, CHUNKS_PER_GRP * P], FP32)
                    for j in range(CHUNKS_PER_GRP):
                        c = g * CHUNKS_PER_GRP + j
                        nc.tensor.matmul(
                            tp_ps[:, j * P:(j + 1) * P],
                            hid_t[:, c * P:(c + 1) * P],
                            ident_sb[:, :],
                            is_transpose=True, start=True, stop=True)
                    hidT_sb = hidT_pool.tile([P, CHUNKS_PER_GRP * P], FP32)
                    # alternate copy engine to balance DVE/ACT load
                    if g % 2 == 0:
                        nc.vector.tensor_copy(hidT_sb[:, :], tp_ps[:, :])
                    else:
                        nc.scalar.copy(hidT_sb[:, :], tp_ps[:, :])
                    for j in range(CHUNKS_PER_GRP):
                        c = g * CHUNKS_PER_GRP + j
                        nc.tensor.matmul(
                            lg_ps[:, :],
                            hidT_sb[:, j * P:(j + 1) * P],
                            gate_sb[:, c * E:(c + 1) * E],
                            start=(c == 0), stop=(c == N_CHUNKS - 1))

                # ---- per-tile tail: softmax / top-2 / stats ----
                logits_sb = out_pool.tile([P, E], FP32)
                nc.vector.tensor_copy(logits_sb[:, :], lg_ps[:, :])

                max_sb = tail_pool.tile([P, 8], FP32)
                nc.vector.max(max_sb[:, :], logits_sb[:, :])
                idx_sb = tail_pool.tile([P, 8], U32)
                nc.vector.max_index(idx_sb[:, :], max_sb[:, :], logits_sb[:, :])

                neg_m0 = tail_pool.tile([P, 1], FP32)
                nc.vector.tensor_scalar_mul(neg_m0[:, :], max_sb[:, 0:1], -1.0)

                exp_sb = tail_pool.tile([P, E], FP32)
                nc.scalar.activation(exp_sb[:, :], logits_sb[:, :],
                                     mybir.ActivationFunctionType.Exp,
                                     bias=neg_m0[:, 0:1], scale=1.0)
                zsum = tail_pool.tile([P, 1], FP32)
                nc.vector.reduce_sum(zsum[:, :], exp_sb[:, :],
                                     axis=mybir.AxisListType.X)
                rz = tail_pool.tile([P, 1], FP32)
                nc.vector.reciprocal(rz[:, :], zsum[:, :])
                probs = tail_pool.tile([P, E], FP32)
                nc.vector.tensor_scalar_mul(probs[:, :], exp_sb[:, :], rz[:, 0:1])
                nc.vector.tensor_add(stat_acc[:, E:2 * E], stat_acc[:, E:2 * E],
                                     probs[:, :])

                idx0_f = tail_pool.tile([P, 1], FP32)
                nc.vector.tensor_copy(idx0_f[:, :], idx_sb[:, 0:1])
                onehot = tail_pool.tile([P, E], FP32)
                nc.vector.tensor_scalar(
                    out=onehot[:, :], in0=iota_sb[:, :], scalar1=idx0_f[:, 0:1],
                    scalar2=None, op0=mybir.AluOpType.is_equal)
                nc.vector.tensor_add(stat_acc[:, 0:E], stat_acc[:, 0:E],
                                     onehot[:, :])

                # routing weights: rw0 = 1/(1+e1), rw1 = e1/(1+e1),
                # e1 = exp(m1 - m0)
                e1 = tail_pool.tile([P, 1], FP32)
                nc.scalar.activation(e1[:, :], max_sb[:, 1:2],
                                     mybir.ActivationFunctionType.Exp,
                                     bias=neg_m0[:, 0:1], scale=1.0)
                den = tail_pool.tile([P, 1], FP32)
                nc.vector.tensor_scalar_add(den[:, :], e1[:, :], 1.0)
                rw_sb = out_pool.tile([P, TOPK], FP32)
                nc.vector.reciprocal(rw_sb[:, 0:1], den[:, :])
                nc.vector.tensor_mul(rw_sb[:, 1:2], e1[:, :], rw_sb[:, 0:1])

                nc.sync.dma_start(logits_out[i * P:(i + 1) * P, :], logits_sb[:, :])
                nc.sync.dma_start(rw_out[i * P:(i + 1) * P, :], rw_sb[:, :])
                nc.sync.dma_start(sel_out[i * P:(i + 1) * P, :],
                                  idx_sb[:, 0:TOPK].bitcast(I32))

            # ---- partition-reduce stats via ones-matmul ----
            st_ps = st_psum.tile([1, 2 * E], FP32)
            nc.tensor.matmul(st_ps[:, :], ones_sb[:, :], stat_acc[:, :],
                             start=True, stop=True)
            st_sb = cpool.tile([1, 2 * E], FP32)
            nc.vector.tensor_copy(st_sb[:, :], st_ps[:, :])
            nc.sync.dma_start(fp_out[:, :], st_sb[:, :])

    nc.compile()
    return nc


_NC = None


def _get_nc():
    global _NC
    if _NC is None:
        _NC = _build()
    return _NC


def run(hidden_states, gate_w, trace=False):
    hidden_states = np.ascontiguousarray(hidden_states, dtype=np.float32)
    gate_w = np.ascontiguousarray(gate_w, dtype=np.float32)
    assert hidden_states.shape == (T, H) and gate_w.shape == (E, H)

    # [c*128+p, e] -> [p, (c e)]
    gate_t = np.ascontiguousarray(
        gate_w.T.reshape(N_CHUNKS, P, E).transpose(1, 0, 2).reshape(P, N_CHUNKS * E))
    ident = np.eye(P, dtype=np.float32)
    iota = np.broadcast_to(np.arange(E, dtype=np.float32), (P, E)).copy()
    ones = np.ones((P, 1), dtype=np.float32)

    in_maps = []
    for i in range(N_CORES):
        in_maps.append({
            "hid": hidden_states[i * T_LOC:(i + 1) * T_LOC],
            "gate_t": gate_t,
            "ident": ident,
            "iota8": iota,
            "ones": ones,
        })

    nc = _get_nc()
    res = bass_utils.run_bass_kernel_spmd(
        nc, in_maps, core_ids=list(range(N_CORES)), trace=trace)

    logits = np.concatenate([res.results[i]["logits_out"] for i in range(N_CORES)], axis=0)
    rw = np.concatenate([res.results[i]["rw_out"] for i in range(N_CORES)], axis=0)
    sel = np.concatenate([res.results[i]["sel_out"] for i in range(N_CORES)], axis=0)
    fp = np.stack([res.results[i]["fp_out"][0] for i in range(N_CORES)], axis=0)

    fp64 = fp.astype(np.float64).sum(axis=0) / T
    f, Pm = fp64[:E], fp64[E:]
    aux = np.float32(AUX_COEF * E * np.sum(f * Pm))

    return (rw, sel, logits, aux), res


def kernel(hidden_states, gate_w):
    (rw, sel, logits, aux), _ = run(hidden_states, gate_w)
    return rw, sel, logits, aux


# revision 7
# speedup vs baseline: 1.3560x; 1.3560x over previous
"""MoE router kernel for TRN2, 8 NeuronCores, data-parallel over tokens.

reference computation (per problem spec, T=16384, H=4096, E=8, K=2):
    logits  = hidden @ gate_w.T            [T, E]
    probs   = softmax(logits, -1)
    rw, sel = top_k(probs, 2); rw /= rw.sum(-1, keepdims=True)
    f = mean(one_hot(sel[:, 0], E)); P = mean(probs, 0)
    aux = 0.01 * E * sum(f * P)
    returns (rw, sel, logits, aux)

Sharding: tokens split 8 ways (2048/core); gate replicated; f/P partial sums
reduced on host (8x16 floats).
"""

import numpy as np

import concourse.bacc as bacc
import concourse.bass as bass
import concourse.mybir as mybir
from concourse import bass_utils
from concourse.tile import TileContext

T, H, E, TOPK = 16384, 4096, 8, 2
AUX_COEF = 0.01
N_CORES = 8
T_LOC = T // N_CORES          # 2048 tokens per core
P = 128                       # partitions
N_TILES = T_LOC // P          # 16 token tiles per core
N_CHUNKS = H // P             # 32 h-chunks
CHUNKS_PER_GRP = 4            # transpose group -> [128, 512] psum bank
N_GRPS = N_CHUNKS // CHUNKS_PER_GRP

FP32 = mybir.dt.float32
F32R = mybir.dt.float32r
U32 = mybir.dt.uint32
I32 = mybir.dt.int32

T_GRP = 512                   # tokens per logitsT matmul group
N_TGRPS = T_LOC // T_GRP      # 4 groups per core
SUBS = T_GRP // P             # 4 token sub-tiles per group


def _tail(nc, tc, pools, lg_src, iota_sb, stat_acc, outs, i):
    """Per-128-token-tile softmax / top-2 / stats / output DMAs.

    lg_src: SBUF [128, 8] fp32 logits for token tile i.
    """
    tail_pool, out_pool = pools
    logits_out, rw_out, sel_out = outs

    max_sb = tail_pool.tile([P, 8], FP32)
    nc.vector.max(max_sb[:, :], lg_src[:, :])
    idx_sb = tail_pool.tile([P, 8], U32)
    nc.vector.max_index(idx_sb[:, :], max_sb[:, :], lg_src[:, :])

    neg_m0 = tail_pool.tile([P, 1], FP32)
    nc.vector.tensor_scalar_mul(neg_m0[:, :], max_sb[:, 0:1], -1.0)

    exp_sb = tail_pool.tile([P, E], FP32)
    nc.scalar.activation(exp_sb[:, :], lg_src[:, :],
                         mybir.ActivationFunctionType.Exp,
                         bias=neg_m0[:, 0:1], scale=1.0)
    zsum = tail_pool.tile([P, 1], FP32)
    nc.vector.reduce_sum(zsum[:, :], exp_sb[:, :], axis=mybir.AxisListType.X)
    rz = tail_pool.tile([P, 1], FP32)
    nc.vector.reciprocal(rz[:, :], zsum[:, :])
    probs = tail_pool.tile([P, E], FP32)
    nc.vector.tensor_scalar_mul(probs[:, :], exp_sb[:, :], rz[:, 0:1])
    nc.vector.tensor_add(stat_acc[:, E:2 * E], stat_acc[:, E:2 * E], probs[:, :])

    idx0_f = tail_pool.tile([P, 1], FP32)
    nc.vector.tensor_copy(idx0_f[:, :], idx_sb[:, 0:1])
    onehot = tail_pool.tile([P, E], FP32)
    nc.vector.tensor_scalar(
        out=onehot[:, :], in0=iota_sb[:, :], scalar1=idx0_f[:, 0:1],
        scalar2=None, op0=mybir.AluOpType.is_equal)
    nc.vector.tensor_add(stat_acc[:, 0:E], stat_acc[:, 0:E], onehot[:, :])

    e1 = tail_pool.tile([P, 1], FP32)
    nc.scalar.activation(e1[:, :], max_sb[:, 1:2],
                         mybir.ActivationFunctionType.Exp,
                         bias=neg_m0[:, 0:1], scale=1.0)
    den = tail_pool.tile([P, 1], FP32)
    nc.vector.tensor_scalar_add(den[:, :], e1[:, :], 1.0)
    rw_sb = out_pool.tile([P, TOPK], FP32)
    nc.vector.reciprocal(rw_sb[:, 0:1], den[:, :])
    nc.vector.tensor_mul(rw_sb[:, 1:2], e1[:, :], rw_sb[:, 0:1])

    nc.sync.dma_start(logits_out[i * P:(i + 1) * P, :], lg_src[:, :])
    nc.sync.dma_start(rw_out[i * P:(i + 1) * P, :], rw_sb[:, :])
    nc.sync.dma_start(sel_out[i * P:(i + 1) * P, :],
                      idx_sb[:, 0:TOPK].bitcast(I32))


def _build():
    nc = bacc.Bacc("TRN2", target_bir_lowering=False, debug=False,
                   num_devices=N_CORES)

    hid = nc.dram_tensor("hid", [T_LOC, H], FP32, kind="ExternalInput")
    # gate pre-arranged on host to [P, N_CHUNKS*E]: [p, (c e)] = gate_w.T[c*128+p, e]
    gate_t = nc.dram_tensor("gate_t", [P, N_CHUNKS * E], FP32, kind="ExternalInput")
    ident = nc.dram_tensor("ident", [P, P], FP32, kind="ExternalInput")
    iota8 = nc.dram_tensor("iota8", [P, E], FP32, kind="ExternalInput")
    ones = nc.dram_tensor("ones", [P, 1], FP32, kind="ExternalInput")

    logits_out = nc.dram_tensor("logits_out", [T_LOC, E], FP32, kind="ExternalOutput")
    rw_out = nc.dram_tensor("rw_out", [T_LOC, TOPK], FP32, kind="ExternalOutput")
    sel_out = nc.dram_tensor("sel_out", [T_LOC, TOPK], I32, kind="ExternalOutput")
    fp_out = nc.dram_tensor("fp_out", [1, 2 * E], FP32, kind="ExternalOutput")

    with TileContext(nc) as tc:
        with (
            tc.tile_pool(name="const", bufs=1) as cpool,
            tc.tile_pool(name="hid", bufs=8) as hid_pool,
            tc.tile_pool(name="hidT", bufs=4) as hidT_pool,
            tc.tile_pool(name="lgT", bufs=2) as lgT_pool,
            tc.tile_pool(name="tail", bufs=4) as tail_pool,
            tc.tile_pool(name="outs", bufs=4) as out_pool,
            tc.tile_pool(name="tp_ps", bufs=2, space="PSUM") as tp_psum,
            tc.tile_pool(name="lg_ps", bufs=2, space="PSUM") as lg_psum,
            tc.tile_pool(name="bt_ps", bufs=2, space="PSUM") as bt_psum,
            tc.tile_pool(name="st_ps", bufs=1, space="PSUM") as st_psum,
        ):
            gate_sb = cpool.tile([P, N_CHUNKS * E], FP32)
            nc.sync.dma_start(gate_sb[:, :], gate_t[:, :])
            ident_sb = cpool.tile([P, P], FP32)
            nc.sync.dma_start(ident_sb[:, :], ident[:, :])
            iota_sb = cpool.tile([P, E], FP32)
            nc.sync.dma_start(iota_sb[:, :], iota8[:, :])
            ones_sb = cpool.tile([P, 1], FP32)
            nc.sync.dma_start(ones_sb[:, :], ones[:, :])

            # stats accumulator: cols 0:8 one-hot(top1) sums, 8:16 prob sums
            stat_acc = cpool.tile([P, 2 * E], FP32)
            nc.vector.memset(stat_acc[:, :], 0.0)

            for grp in range(N_TGRPS):
                hid_ts = []
                for s in range(SUBS):
                    ht = hid_pool.tile([P, H], FP32, tag="hid")
                    row0 = (grp * SUBS + s) * P
                    nc.sync.dma_start(ht[:, :], hid[row0:row0 + P, :])
                    hid_ts.append(ht)

                # logitsT accumulation: [8, 512] over 32 h-chunks (f32r)
                lgT_ps = lg_psum.tile([8, T_GRP], FP32)
                for c in range(N_CHUNKS):
                    tp_ps = tp_psum.tile([P, T_GRP], FP32)
                    for s in range(SUBS):
                        nc.tensor.matmul(
                            tp_ps[:, s * P:(s + 1) * P],
                            hid_ts[s][:, c * P:(c + 1) * P],
                            ident_sb[:, :],
                            is_transpose=True, start=True, stop=True)
                    hidT_sb = hidT_pool.tile([P, T_GRP], FP32)
                    # alternate copy engine to balance DVE/ACT load
                    if c % 2 == 0:
                        nc.vector.tensor_copy(hidT_sb[:, :], tp_ps[:, :])
                    else:
                        nc.scalar.copy(hidT_sb[:, :], tp_ps[:, :])
                    nc.tensor.matmul(
                        lgT_ps[:, :],
                        gate_sb[:, c * E:(c + 1) * E],
                        hidT_sb[:, :],
                        start=(c == 0), stop=(c == N_CHUNKS - 1))

                lgT_sb = lgT_pool.tile([8, T_GRP], FP32)
                nc.vector.tensor_copy(lgT_sb[:, :], lgT_ps[:, :])

                for s in range(SUBS):
                    # back-transpose [8, 128] -> [128, 8]
                    bt_ps = bt_psum.tile([P, E], FP32)
                    nc.tensor.matmul(
                        bt_ps[:, :],
                        lgT_sb[:, s * P:(s + 1) * P],
                        ident_sb[0:E, 0:E],
                        is_transpose=True, start=True, stop=True)
                    logits_sb = out_pool.tile([P, E], FP32)
                    nc.vector.tensor_copy(logits_sb[:, :], bt_ps[:, :])
                    _tail(nc, tc, (tail_pool, out_pool), logits_sb, iota_sb,
                          stat_acc, (logits_out, rw_out, sel_out),
                          grp * SUBS + s)

            # ---- partition-reduce stats via ones-matmul ----
            st_ps = st_psum.tile([1, 2 * E], FP32)
            nc.tensor.matmul(st_ps[:, :], ones_sb[:, :], stat_acc[:, :],
                             start=True, stop=True)
            st_sb = cpool.tile([1, 2 * E], FP32)
            nc.vector.tensor_copy(st_sb[:, :], st_ps[:, :])
            nc.sync.dma_start(fp_out[:, :], st_sb[:, :])

    nc.compile()
    return nc


_NC = None


def _get_nc():
    global _NC
    if _NC is None:
        _NC = _build()
    return _NC


def run(hidden_states, gate_w, trace=False):
    hidden_states = np.ascontiguousarray(hidden_states, dtype=np.float32)
    gate_w = np.ascontiguousarray(gate_w, dtype=np.float32)
    assert hidden_states.shape == (T, H) and gate_w.shape == (E, H)

    # [c*128+p, e] -> [p, (c e)]
    gate_t = np.ascontiguousarray(
        gate_w.T.reshape(N_CHUNKS, P, E).transpose(1, 0, 2).reshape(P, N_CHUNKS * E))
    ident = np.eye(P, dtype=np.float32)
    iota = np.broadcast_to(np.arange(E, dtype=np.float32), (P, E)).copy()
    ones = np.ones((P, 1), dtype=np.float32)

    in_maps = []
    for i in range(N_CORES):
        in_maps.append({
            "hid": hidden_states[i * T_LOC:(i + 1) * T_LOC],
            "gate_t": gate_t,
            "ident": ident,
            "iota8": iota,
            "ones": ones,
        })

    nc = _get_nc()
    res = bass_utils.run_bass_kernel_spmd(
        nc, in_maps, core_ids=list(range(N_CORES)), trace=trace)

    logits = np.concatenate([res.results[i]["logits_out"] for i in range(N_CORES)], axis=0)
    rw = np.concatenate([res.results[i]["rw_out"] for i in range(N_CORES)], axis=0)
    sel = np.concatenate([res.results[i]["sel_out"] for i in range(N_CORES)], axis=0)
    fp = np.stack([res.results[i]["fp_out"][0] for i in range(N_CORES)], axis=0)

    fp64 = fp.astype(np.float64).sum(axis=0) / T
    f, Pm = fp64[:E], fp64[E:]
    aux = np.float32(AUX_COEF * E * np.sum(f * Pm))

    return (rw, sel, logits, aux), res


def kernel(hidden_states, gate_w):
    (rw, sel, logits, aux), _ = run(hidden_states, gate_w)
    return rw, sel, logits, aux


# revision 11
# speedup vs baseline: 1.5586x; 1.1494x over previous
"""MoE router kernel for TRN2, 8 NeuronCores, data-parallel over tokens.

reference computation (per problem spec, T=16384, H=4096, E=8, K=2):
    logits  = hidden @ gate_w.T            [T, E]
    probs   = softmax(logits, -1)
    rw, sel = top_k(probs, 2); rw /= rw.sum(-1, keepdims=True)
    f = mean(one_hot(sel[:, 0], E)); P = mean(probs, 0)
    aux = 0.01 * E * sum(f * P)
    returns (rw, sel, logits, aux)

Sharding: tokens split 8 ways (2048/core); gate replicated; f/P partial sums
reduced on host (8x16 floats).
"""

import numpy as np

import concourse.bacc as bacc
import concourse.bass as bass
import concourse.mybir as mybir
from concourse import bass_utils
from concourse.tile import TileContext

T, H, E, TOPK = 16384, 4096, 8, 2
AUX_COEF = 0.01
N_CORES = 8
T_LOC = T // N_CORES          # 2048 tokens per core
P = 128                       # partitions
N_TILES = T_LOC // P          # 16 token tiles per core
N_CHUNKS = H // P             # 32 h-chunks
CHUNKS_PER_GRP = 4            # transpose group -> [128, 512] psum bank
N_GRPS = N_CHUNKS // CHUNKS_PER_GRP

FP32 = mybir.dt.float32
F32R = mybir.dt.float32r
U32 = mybir.dt.uint32
I32 = mybir.dt.int32

T_GRP = 512                   # tokens per logitsT matmul group
N_TGRPS = T_LOC // T_GRP      # 4 groups per core
SUBS = T_GRP // P             # 4 token sub-tiles per group


def _tail(nc, tc, pools, lg_src, iota_sb, stat_acc, outs, i):
    """Per-128-token-tile softmax / top-2 / stats / output DMAs.

    lg_src: SBUF [128, 8] fp32 logits for token tile i.
    """
    tail_pool, out_pool = pools
    logits_out, rw_out, sel_out = outs

    max_sb = tail_pool.tile([P, 8], FP32)
    nc.vector.max(max_sb[:, :], lg_src[:, :])
    idx_sb = tail_pool.tile([P, 8], U32)
    nc.vector.max_index(idx_sb[:, :], max_sb[:, :], lg_src[:, :])

    neg_m0 = tail_pool.tile([P, 1], FP32)
    nc.vector.tensor_scalar_mul(neg_m0[:, :], max_sb[:, 0:1], -1.0)

    exp_sb = tail_pool.tile([P, E], FP32)
    nc.scalar.activation(exp_sb[:, :], lg_src[:, :],
                         mybir.ActivationFunctionType.Exp,
                         bias=neg_m0[:, 0:1], scale=1.0)
    zsum = tail_pool.tile([P, 1], FP32)
    nc.vector.reduce_sum(zsum[:, :], exp_sb[:, :], axis=mybir.AxisListType.X)
    rz = tail_pool.tile([P, 1], FP32)
    nc.vector.reciprocal(rz[:, :], zsum[:, :])
    probs = tail_pool.tile([P, E], FP32)
    nc.vector.tensor_scalar_mul(probs[:, :], exp_sb[:, :], rz[:, 0:1])
    nc.vector.tensor_add(stat_acc[:, E:2 * E], stat_acc[:, E:2 * E], probs[:, :])

    idx0_f = tail_pool.tile([P, 1], FP32)
    nc.vector.tensor_copy(idx0_f[:, :], idx_sb[:, 0:1])
    onehot = tail_pool.tile([P, E], FP32)
    nc.vector.tensor_scalar(
        out=onehot[:, :], in0=iota_sb[:, :], scalar1=idx0_f[:, 0:1],
        scalar2=None, op0=mybir.AluOpType.is_equal)
    nc.vector.tensor_add(stat_acc[:, 0:E], stat_acc[:, 0:E], onehot[:, :])

    e1 = tail_pool.tile([P, 1], FP32)
    nc.scalar.activation(e1[:, :], max_sb[:, 1:2],
                         mybir.ActivationFunctionType.Exp,
                         bias=neg_m0[:, 0:1], scale=1.0)
    den = tail_pool.tile([P, 1], FP32)
    nc.vector.tensor_scalar_add(den[:, :], e1[:, :], 1.0)
    rw_sb = out_pool.tile([P, TOPK], FP32)
    nc.vector.reciprocal(rw_sb[:, 0:1], den[:, :])
    nc.vector.tensor_mul(rw_sb[:, 1:2], e1[:, :], rw_sb[:, 0:1])

    nc.sync.dma_start(logits_out[i * P:(i + 1) * P, :], lg_src[:, :])
    nc.sync.dma_start(rw_out[i * P:(i + 1) * P, :], rw_sb[:, :])
    nc.sync.dma_start(sel_out[i * P:(i + 1) * P, :],
                      idx_sb[:, 0:TOPK].bitcast(I32))


def _build():
    nc = bacc.Bacc("TRN2", target_bir_lowering=False, debug=False,
                   num_devices=N_CORES)

    hid = nc.dram_tensor("hid", [T_LOC, H], FP32, kind="ExternalInput")
    # gate pre-arranged on host to [P, N_CHUNKS*E]: [p, (c e)] = gate_w.T[c*128+p, e]
    gate_t = nc.dram_tensor("gate_t", [P, N_CHUNKS * E], FP32, kind="ExternalInput")
    ident = nc.dram_tensor("ident", [P, P], FP32, kind="ExternalInput")
    iota8 = nc.dram_tensor("iota8", [P, E], FP32, kind="ExternalInput")
    ones = nc.dram_tensor("ones", [P, 1], FP32, kind="ExternalInput")

    logits_out = nc.dram_tensor("logits_out", [T_LOC, E], FP32, kind="ExternalOutput")
    rw_out = nc.dram_tensor("rw_out", [T_LOC, TOPK], FP32, kind="ExternalOutput")
    sel_out = nc.dram_tensor("sel_out", [T_LOC, TOPK], I32, kind="ExternalOutput")
    fp_out = nc.dram_tensor("fp_out", [1, 2 * E], FP32, kind="ExternalOutput")

    with TileContext(nc) as tc:
        with (
            tc.tile_pool(name="const", bufs=1) as cpool,
            tc.tile_pool(name="hid", bufs=8) as hid_pool,
            tc.tile_pool(name="hidT", bufs=6) as hidT_pool,
            tc.tile_pool(name="lgT", bufs=2) as lgT_pool,
            tc.tile_pool(name="tail", bufs=4) as tail_pool,
            tc.tile_pool(name="outs", bufs=4) as out_pool,
            tc.tile_pool(name="tp_ps", bufs=4, space="PSUM") as tp_psum,
            tc.tile_pool(name="lg_ps", bufs=2, space="PSUM") as lg_psum,
            tc.tile_pool(name="bt_ps", bufs=1, space="PSUM") as bt_psum,
        ):
            gate_sb = cpool.tile([P, N_CHUNKS * E], FP32)
            nc.sync.dma_start(gate_sb[:, :], gate_t[:, :])
            ident_sb = cpool.tile([P, P], FP32)
            nc.sync.dma_start(ident_sb[:, :], ident[:, :])
            iota_sb = cpool.tile([P, E], FP32)
            nc.sync.dma_start(iota_sb[:, :], iota8[:, :])
            ones_sb = cpool.tile([P, 1], FP32)
            nc.sync.dma_start(ones_sb[:, :], ones[:, :])

            # stats accumulator: cols 0:8 one-hot(top1) sums, 8:16 prob sums
            stat_acc = cpool.tile([P, 2 * E], FP32)
            nc.vector.memset(stat_acc[:, :], 0.0)

            for grp in range(N_TGRPS):
                hid_ts = []
                for s in range(SUBS):
                    ht = hid_pool.tile([P, H], FP32, tag="hid")
                    row0 = (grp * SUBS + s) * P
                    nc.sync.dma_start(ht[:, :], hid[row0:row0 + P, :])
                    hid_ts.append(ht)

                # logitsT accumulation: [8, 512] over 32 h-chunks (f32r)
                lgT_ps = lg_psum.tile([8, T_GRP], FP32)
                for c in range(N_CHUNKS):
                    tp_ps = tp_psum.tile([P, T_GRP], FP32)
                    for s in range(SUBS):
                        nc.tensor.matmul(
                            tp_ps[:, s * P:(s + 1) * P],
                            hid_ts[s][:, c * P:(c + 1) * P],
                            ident_sb[:, :],
                            is_transpose=True, start=True, stop=True)
                    hidT_sb = hidT_pool.tile([P, T_GRP], FP32)
                    # alternate copy engine to balance DVE/ACT load
                    if c % 2 == 0:
                        nc.vector.tensor_copy(hidT_sb[:, :], tp_ps[:, :])
                    else:
                        nc.scalar.copy(hidT_sb[:, :], tp_ps[:, :])
                    nc.tensor.matmul(
                        lgT_ps[:, :],
                        gate_sb[:, c * E:(c + 1) * E],
                        hidT_sb[:, :],
                        start=(c == 0), stop=(c == N_CHUNKS - 1))

                lgT_sb = lgT_pool.tile([8, T_GRP], FP32)
                nc.vector.tensor_copy(lgT_sb[:, :], lgT_ps[:, :])

                for s in range(SUBS):
                    # back-transpose [8, 128] -> [128, 8]
                    bt_ps = bt_psum.tile([P, E], FP32)
                    nc.tensor.matmul(
                        bt_ps[:, :],
                        lgT_sb[:, s * P:(s + 1) * P],
                        ident_sb[0:E, 0:E],
                        is_transpose=True, start=True, stop=True)
                    logits_sb = out_pool.tile([P, E], FP32)
                    nc.vector.tensor_copy(logits_sb[:, :], bt_ps[:, :])
                    _tail(nc, tc, (tail_pool, out_pool), logits_sb, iota_sb,
                          stat_acc, (logits_out, rw_out, sel_out),
                          grp * SUBS + s)

            # ---- partition-reduce stats via ones-matmul ----
            st_ps = bt_psum.tile([1, 2 * E], FP32, tag="st", bufs=1)
            nc.tensor.matmul(st_ps[:, :], ones_sb[:, :], stat_acc[:, :],
                             start=True, stop=True)
            st_sb = cpool.tile([1, 2 * E], FP32)
            nc.vector.tensor_copy(st_sb[:, :], st_ps[:, :])
            nc.sync.dma_start(fp_out[:, :], st_sb[:, :])

    nc.compile()
    return nc


_NC = None


def _get_nc():
    global _NC
    if _NC is None:
        _NC = _build()
    return _NC


def run(hidden_states, gate_w, trace=False):
    hidden_states = np.ascontiguousarray(hidden_states, dtype=np.float32)
    gate_w = np.ascontiguousarray(gate_w, dtype=np.float32)
    assert hidden_states.shape == (T, H) and gate_w.shape == (E, H)

    # [c*128+p, e] -> [p, (c e)]
    gate_t = np.ascontiguousarray(
        gate_w.T.reshape(N_CHUNKS, P, E).transpose(1, 0, 2).reshape(P, N_CHUNKS * E))
    ident = np.eye(P, dtype=np.float32)
    iota = np.broadcast_to(np.arange(E, dtype=np.float32), (P, E)).copy()
    ones = np.ones((P, 1), dtype=np.float32)

    in_maps = []
    for i in range(N_CORES):
        in_maps.append({
            "hid": hidden_states[i * T_LOC:(i + 1) * T_LOC],
            "gate_t": gate_t,
            "ident": ident,
            "iota8": iota,
            "ones": ones,
        })

    nc = _get_nc()
    res = bass_utils.run_bass_kernel_spmd(
        nc, in_maps, core_ids=list(range(N_CORES)), trace=trace)

    logits = np.concatenate([res.results[i]["logits_out"] for i in range(N_CORES)], axis=0)
    rw = np.concatenate([res.results[i]["rw_out"] for i in range(N_CORES)], axis=0)
    sel = np.concatenate([res.results[i]["sel_out"] for i in range(N_CORES)], axis=0)
    fp = np.stack([res.results[i]["fp_out"][0] for i in range(N_CORES)], axis=0)

    fp64 = fp.astype(np.float64).sum(axis=0) / T
    f, Pm = fp64[:E], fp64[E:]
    aux = np.float32(AUX_COEF * E * np.sum(f * Pm))

    return (rw, sel, logits, aux), res


def kernel(hidden_states, gate_w):
    (rw, sel, logits, aux), _ = run(hidden_states, gate_w)
    return rw, sel, logits, aux


# revision 13
# speedup vs baseline: 1.6467x; 1.0566x over previous
"""MoE router kernel for TRN2, 8 NeuronCores, data-parallel over tokens.

reference computation (per problem spec, T=16384, H=4096, E=8, K=2):
    logits  = hidden @ gate_w.T            [T, E]
    probs   = softmax(logits, -1)
    rw, sel = top_k(probs, 2); rw /= rw.sum(-1, keepdims=True)
    f = mean(one_hot(sel[:, 0], E)); P = mean(probs, 0)
    aux = 0.01 * E * sum(f * P)
    returns (rw, sel, logits, aux)

Sharding: tokens split 8 ways (2048/core); gate replicated; f/P partial sums
reduced on host (8x16 floats).
"""

import numpy as np

import concourse.bacc as bacc
import concourse.bass as bass
import concourse.mybir as mybir
from concourse import bass_utils
from concourse.tile import TileContext

T, H, E, TOPK = 16384, 4096, 8, 2
AUX_COEF = 0.01
N_CORES = 8
T_LOC = T // N_CORES          # 2048 tokens per core
P = 128                       # partitions
N_TILES = T_LOC // P          # 16 token tiles per core
N_CHUNKS = H // P             # 32 h-chunks
CHUNKS_PER_GRP = 4            # transpose group -> [128, 512] psum bank
N_GRPS = N_CHUNKS // CHUNKS_PER_GRP

FP32 = mybir.dt.float32
F32R = mybir.dt.float32r
U32 = mybir.dt.uint32
I32 = mybir.dt.int32

T_GRP = 512                   # tokens per logitsT matmul group
N_TGRPS = T_LOC // T_GRP      # 4 groups per core
SUBS = T_GRP // P             # 4 token sub-tiles per group


def _tail(nc, tc, pools, lg_src, iota_sb, stat_acc, outs, i):
    """Per-128-token-tile softmax / top-2 / stats / output DMAs.

    lg_src: SBUF [128, 8] fp32 logits for token tile i.
    """
    tail_pool, out_pool = pools
    logits_out, rw_out, sel_out = outs

    max_sb = tail_pool.tile([P, 8], FP32)
    nc.vector.max(max_sb[:, :], lg_src[:, :])
    idx_sb = tail_pool.tile([P, 8], U32)
    nc.vector.max_index(idx_sb[:, :], max_sb[:, :], lg_src[:, :])

    neg_m0 = tail_pool.tile([P, 1], FP32)
    nc.vector.tensor_scalar_mul(neg_m0[:, :], max_sb[:, 0:1], -1.0)

    exp_sb = tail_pool.tile([P, E], FP32)
    nc.scalar.activation(exp_sb[:, :], lg_src[:, :],
                         mybir.ActivationFunctionType.Exp,
                         bias=neg_m0[:, 0:1], scale=1.0)
    zsum = tail_pool.tile([P, 1], FP32)
    nc.vector.reduce_sum(zsum[:, :], exp_sb[:, :], axis=mybir.AxisListType.X)
    rz = tail_pool.tile([P, 1], FP32)
    nc.vector.reciprocal(rz[:, :], zsum[:, :])
    probs = tail_pool.tile([P, E], FP32)
    nc.vector.tensor_scalar_mul(probs[:, :], exp_sb[:, :], rz[:, 0:1])
    nc.vector.tensor_add(stat_acc[:, E:2 * E], stat_acc[:, E:2 * E], probs[:, :])

    idx0_f = tail_pool.tile([P, 1], FP32)
    nc.vector.tensor_copy(idx0_f[:, :], idx_sb[:, 0:1])
    onehot = tail_pool.tile([P, E], FP32)
    nc.vector.tensor_scalar(
        out=onehot[:, :], in0=iota_sb[:, :], scalar1=idx0_f[:, 0:1],
        scalar2=None, op0=mybir.AluOpType.is_equal)
    nc.vector.tensor_add(stat_acc[:, 0:E], stat_acc[:, 0:E], onehot[:, :])

    e1 = tail_pool.tile([P, 1], FP32)
    nc.scalar.activation(e1[:, :], max_sb[:, 1:2],
                         mybir.ActivationFunctionType.Exp,
                         bias=neg_m0[:, 0:1], scale=1.0)
    den = tail_pool.tile([P, 1], FP32)
    nc.vector.tensor_scalar_add(den[:, :], e1[:, :], 1.0)
    rw_sb = out_pool.tile([P, TOPK], FP32)
    nc.vector.reciprocal(rw_sb[:, 0:1], den[:, :])
    nc.vector.tensor_mul(rw_sb[:, 1:2], e1[:, :], rw_sb[:, 0:1])

    nc.sync.dma_start(logits_out[i * P:(i + 1) * P, :], lg_src[:, :])
    nc.sync.dma_start(rw_out[i * P:(i + 1) * P, :], rw_sb[:, :])
    nc.sync.dma_start(sel_out[i * P:(i + 1) * P, :],
                      idx_sb[:, 0:TOPK].bitcast(I32))


def _build():
    nc = bacc.Bacc("TRN2", target_bir_lowering=False, debug=False,
                   num_devices=N_CORES)

    hid = nc.dram_tensor("hid", [T_LOC, H], FP32, kind="ExternalInput")
    # gate pre-arranged on host to [P, N_CHUNKS*E]: [p, (c e)] = gate_w.T[c*128+p, e]
    gate_t = nc.dram_tensor("gate_t", [P, N_CHUNKS * E], FP32, kind="ExternalInput")
    ident = nc.dram_tensor("ident", [P, P], FP32, kind="ExternalInput")
    iota8 = nc.dram_tensor("iota8", [P, E], FP32, kind="ExternalInput")
    ones = nc.dram_tensor("ones", [P, 1], FP32, kind="ExternalInput")

    logits_out = nc.dram_tensor("logits_out", [T_LOC, E], FP32, kind="ExternalOutput")
    rw_out = nc.dram_tensor("rw_out", [T_LOC, TOPK], FP32, kind="ExternalOutput")
    sel_out = nc.dram_tensor("sel_out", [T_LOC, TOPK], I32, kind="ExternalOutput")
    fp_out = nc.dram_tensor("fp_out", [1, 2 * E], FP32, kind="ExternalOutput")

    with TileContext(nc) as tc:
        with (
            tc.tile_pool(name="const", bufs=1) as cpool,
            tc.tile_pool(name="hid", bufs=8) as hid_pool,
            tc.tile_pool(name="hidT", bufs=8) as hidT_pool,
            tc.tile_pool(name="lgT", bufs=2) as lgT_pool,
            tc.tile_pool(name="tail", bufs=4) as tail_pool,
            tc.tile_pool(name="outs", bufs=4) as out_pool,
            tc.tile_pool(name="tp_ps", bufs=4, space="PSUM") as tp_psum,
            tc.tile_pool(name="lg_ps", bufs=2, space="PSUM") as lg_psum,
            tc.tile_pool(name="bt_ps", bufs=1, space="PSUM") as bt_psum,
        ):
            gate_sb = cpool.tile([P, N_CHUNKS * E], FP32)
            nc.sync.dma_start(gate_sb[:, :], gate_t[:, :])
            ident_sb = cpool.tile([P, P], FP32)
            nc.sync.dma_start(ident_sb[:, :], ident[:, :])
            iota_sb = cpool.tile([P, E], FP32)
            nc.sync.dma_start(iota_sb[:, :], iota8[:, :])
            ones_sb = cpool.tile([P, 1], FP32)
            nc.sync.dma_start(ones_sb[:, :], ones[:, :])

            # stats accumulator: cols 0:8 one-hot(top1) sums, 8:16 prob sums
            stat_acc = cpool.tile([P, 2 * E], FP32)
            nc.vector.memset(stat_acc[:, :], 0.0)

            H_SLC = H // 4
            for grp in range(N_TGRPS):
                hid_ts = []
                for s in range(SUBS):
                    ht = hid_pool.tile([P, H], FP32, tag="hid")
                    hid_ts.append(ht)
                # column-sliced, slice-major issue order: the chunk-c
                # transposes only need slice c//8 of each sub-tile, so the
                # first transposes start ~4x sooner.
                for j in range(4):
                    for s in range(SUBS):
                        row0 = (grp * SUBS + s) * P
                        nc.sync.dma_start(
                            hid_ts[s][:, j * H_SLC:(j + 1) * H_SLC],
                            hid[row0:row0 + P, j * H_SLC:(j + 1) * H_SLC])

                # logitsT accumulation: [8, 512] over 32 h-chunks (f32r)
                lgT_ps = lg_psum.tile([8, T_GRP], FP32)
                for c in range(N_CHUNKS):
                    tp_ps = tp_psum.tile([P, T_GRP], FP32)
                    for s in range(SUBS):
                        nc.tensor.matmul(
                            tp_ps[:, s * P:(s + 1) * P],
                            hid_ts[s][:, c * P:(c + 1) * P],
                            ident_sb[:, :],
                            is_transpose=True, start=True, stop=True)
                    hidT_sb = hidT_pool.tile([P, T_GRP], FP32)
                    # alternate copy engine to balance DVE/ACT load
                    if c % 2 == 0:
                        nc.vector.tensor_copy(hidT_sb[:, :], tp_ps[:, :])
                    else:
                        nc.scalar.copy(hidT_sb[:, :], tp_ps[:, :])
                    nc.tensor.matmul(
                        lgT_ps[:, :],
                        gate_sb[:, c * E:(c + 1) * E],
                        hidT_sb[:, :],
                        start=(c == 0), stop=(c == N_CHUNKS - 1))

                lgT_sb = lgT_pool.tile([8, T_GRP], FP32)
                nc.vector.tensor_copy(lgT_sb[:, :], lgT_ps[:, :])

                for s in range(SUBS):
                    # back-transpose [8, 128] -> [128, 8]
                    bt_ps = bt_psum.tile([P, E], FP32)
                    nc.tensor.matmul(
                        bt_ps[:, :],
                        lgT_sb[:, s * P:(s + 1) * P],
                        ident_sb[0:E, 0:E],
                        is_transpose=True, start=True, stop=True)
                    logits_sb = out_pool.tile([P, E], FP32)
                    nc.vector.tensor_copy(logits_sb[:, :], bt_ps[:, :])
                    _tail(nc, tc, (tail_pool, out_pool), logits_sb, iota_sb,
                          stat_acc, (logits_out, rw_out, sel_out),
                          grp * SUBS + s)

            # ---- partition-reduce stats via ones-matmul ----
            st_ps = bt_psum.tile([1, 2 * E], FP32, tag="st", bufs=1)
            nc.tensor.matmul(st_ps[:, :], ones_sb[:, :], stat_acc[:, :],
                             start=True, stop=True)
            st_sb = cpool.tile([1, 2 * E], FP32)
            nc.vector.tensor_copy(st_sb[:, :], st_ps[:, :])
            nc.sync.dma_start(fp_out[:, :], st_sb[:, :])

    nc.compile()
    return nc


_NC = None


def _get_nc():
    global _NC
    if _NC is None:
        _NC = _build()
    return _NC


def run(hidden_states, gate_w, trace=False):
    hidden_states = np.ascontiguousarray(hidden_states, dtype=np.float32)
    gate_w = np.ascontiguousarray(gate_w, dtype=np.float32)
    assert hidden_states.shape == (T, H) and gate_w.shape == (E, H)

    # [c*128+p, e] -> [p, (c e)]
    gate_t = np.ascontiguousarray(
        gate_w.T.reshape(N_CHUNKS, P, E).transpose(1, 0, 2).reshape(P, N_CHUNKS * E))
    ident = np.eye(P, dtype=np.float32)
    iota = np.broadcast_to(np.arange(E, dtype=np.float32), (P, E)).copy()
    ones = np.ones((P, 1), dtype=np.float32)

    in_maps = []
    for i in range(N_CORES):
        in_maps.append({
            "hid": hidden_states[i * T_LOC:(i + 1) * T_LOC],
            "gate_t": gate_t,
            "ident": ident,
            "iota8": iota,
            "ones": ones,
        })

    nc = _get_nc()
    res = bass_utils.run_bass_kernel_spmd(
        nc, in_maps, core_ids=list(range(N_CORES)), trace=trace)

    logits = np.concatenate([res.results[i]["logits_out"] for i in range(N_CORES)], axis=0)
    rw = np.concatenate([res.results[i]["rw_out"] for i in range(N_CORES)], axis=0)
    sel = np.concatenate([res.results[i]["sel_out"] for i in range(N_CORES)], axis=0)
    fp = np.stack([res.results[i]["fp_out"][0] for i in range(N_CORES)], axis=0)

    fp64 = fp.astype(np.float64).sum(axis=0) / T
    f, Pm = fp64[:E], fp64[E:]
    aux = np.float32(AUX_COEF * E * np.sum(f * Pm))

    return (rw, sel, logits, aux), res


def kernel(hidden_states, gate_w):
    (rw, sel, logits, aux), _ = run(hidden_states, gate_w)
    return rw, sel, logits, aux
